# revision 1
# baseline (speedup 1.0000x reference)
"""Document-causal GQA attention on 8 TRN2 NeuronCores.

Strategy: the packed-document mask makes attention block-diagonal over
(batch, document) segments, so each of the 8 cores gets one segment's
queries (2 batches x ~4 docs) together with its KV window — no
cross-core communication at all. The host shards/transposes inputs,
each core runs the full QKV->RoPE->softmax->PV->Wo pipeline on its
rows, and the host scatters the disjoint output rows back.

Device kernel (SPMD, one graph): bf16 matmuls (FWL-eligible 128x128
stationary operands via a head permutation + zero-padded K/V weights),
fp32 PSUM, ACT exp with folded 1/sqrt(hd) scale, diagonal-block-only
masking in the pure-causal case, softmax denominators via a ones
column appended to V, batched reciprocal + ones-outer-product
broadcast for normalization.
"""
import numpy as np
import ml_dtypes

from contextlib import ExitStack

import concourse.bass as bass
import concourse.tile as tile
from concourse import bacc, mybir
from concourse.bass_utils import run_bass_kernel_spmd

BS, S, D, H, KVH, HD = 2, 2048, 2048, 32, 8, 64
N_REP = H // KVH
HQ = H * HD
HKV = KVH * HD
P = 128
N_CORES = 8
DT = D // P
HC = HQ // 512
HQT = HQ // P

f32 = mybir.dt.float32
bf16 = mybir.dt.bfloat16
EXPF = mybir.ActivationFunctionType.Exp
bf = ml_dtypes.bfloat16

HEAD_ORDER = [i // 2 if i % 2 == 0 else 16 + i // 2 for i in range(32)]


# ---------------------------------------------------------------------------
# host-side planning
# ---------------------------------------------------------------------------

def _round_up(x, m):
    return ((x + m - 1) // m) * m


def _plan_jobs(sequence_id):
    jobs = []
    for b in range(BS):
        sid = np.asarray(sequence_id[b])
        starts = [0] + list(np.where(np.diff(sid) != 0)[0] + 1) + [len(sid)]
        for i in range(len(starts) - 1):
            jobs.append([b, int(starts[i]), int(starts[i + 1] - starts[i]),
                         int(starts[i])])
    while len(jobs) > N_CORES:
        best, bi = None, -1
        for i in range(len(jobs) - 1):
            a, c = jobs[i], jobs[i + 1]
            if a[0] == c[0] and a[1] + a[2] == c[1]:
                cost = (c[1] + c[2]) - min(a[3], c[3])
                if best is None or cost < best:
                    best, bi = cost, i
        a, c = jobs[bi], jobs[bi + 1]
        jobs[bi] = [a[0], a[1], a[2] + c[2], min(a[3], c[3])]
        del jobs[bi + 1]
    while len(jobs) < N_CORES:
        i = max(range(len(jobs)), key=lambda j: jobs[j][2])
        b, qs, ql, ks = jobs[i]
        if ql < 2:
            jobs.append([b, qs, 0, qs])
            continue
        h = ql // 2
        jobs[i] = [b, qs, h, ks]
        jobs.insert(i + 1, [b, qs + h, ql - h, ks])
    return jobs


def _permute_wq(wq_t):
    return np.ascontiguousarray(
        wq_t.reshape(D, 32, 64)[:, HEAD_ORDER, :].reshape(D, HQ))


def _permute_wo(wo_t):
    return np.ascontiguousarray(
        wo_t.reshape(32, 64, D)[HEAD_ORDER].reshape(HQ, D))


def _core_inputs(job, NQ, NK, x, sequence_id, cos_tab, sin_tab):
    b, qs, ql, ks = job
    kl = qs + ql - ks

    xq_t = np.zeros((D, NQ), dtype=np.float32)
    xq_t[:, :ql] = x[b, qs:qs + ql].T
    xk_t = np.zeros((D, NK), dtype=np.float32)
    xk_t[:, :kl] = x[b, ks:ks + kl].T

    def rope(start, ln, n):
        cos = np.ones((n, 64), dtype=np.float32)
        sin = np.zeros((n, 64), dtype=np.float32)
        c = cos_tab[start:start + ln]
        s = sin_tab[start:start + ln]
        cos[:ln, 0::2] = c
        cos[:ln, 1::2] = c
        sin[:ln, 0::2] = -s
        sin[:ln, 1::2] = s
        return (np.tile(cos, (1, 8)).astype(bf), np.tile(sin, (1, 8)).astype(bf))

    cos_q, sin_q = rope(qs, ql, NQ)
    cos_k, sin_k = rope(ks, kl, NK)

    sid = np.asarray(sequence_id[b])
    sid_q = np.full(NQ, -2, dtype=np.int64)
    sid_q[:ql] = sid[qs:qs + ql]
    sid_k = np.full(NK, -1, dtype=np.int64)
    sid_k[:kl] = sid[ks:ks + kl]
    gq = qs + np.arange(NQ)
    gk = ks + np.arange(NK)
    mask = ((sid_k[:, None] == sid_q[None, :]) &
            (gk[:, None] <= gq[None, :])).astype(np.float32)
    # padded query columns attend to key 0 so denominators stay finite
    mask[0, ql:] = 1.0
    kones = np.zeros((NK, 1), dtype=np.float32)
    kones[:kl] = 1.0

    return {
        "xq_t": xq_t.astype(bf), "xk_t": xk_t.astype(bf),
        "cos_q": cos_q, "sin_q": sin_q, "cos_k": cos_k, "sin_k": sin_k,
        "maskm": mask.astype(bf), "kones": kones.astype(bf),
    }


# ---------------------------------------------------------------------------
# device graph
# ---------------------------------------------------------------------------

_BUILD_CACHE = {}


def _build(NQ, NK, offs_max, causal):
    key = (NQ, NK, offs_max, causal)
    if key in _BUILD_CACHE:
        return _BUILD_CACHE[key]
    NQT, NKT = NQ // P, NK // P
    qchunks = [(c * 512, min(512, NQ - c * 512)) for c in range((NQ + 511) // 512)]

    nc = bacc.Bacc("TRN2", target_bir_lowering=False, debug=False,
                   num_devices=N_CORES)

    xq_d = nc.dram_tensor("xq_t", [D, NQ], bf16, kind="ExternalInput").ap()
    xk_d = nc.dram_tensor("xk_t", [D, NK], bf16, kind="ExternalInput").ap()
    wq_d = nc.dram_tensor("wq_t", [D, HQ], bf16, kind="ExternalInput").ap()
    wk_d = nc.dram_tensor("wk_t", [D, HKV], bf16, kind="ExternalInput").ap()
    wv_d = nc.dram_tensor("wv_t", [D, HKV], bf16, kind="ExternalInput").ap()
    wo_d = nc.dram_tensor("wo_t", [HQ, D], bf16, kind="ExternalInput").ap()
    cosq_d = nc.dram_tensor("cos_q", [NQ, 512], bf16, kind="ExternalInput").ap()
    sinq_d = nc.dram_tensor("sin_q", [NQ, 512], bf16, kind="ExternalInput").ap()
    cosk_d = nc.dram_tensor("cos_k", [NK, 512], bf16, kind="ExternalInput").ap()
    sink_d = nc.dram_tensor("sin_k", [NK, 512], bf16, kind="ExternalInput").ap()
    mask_d = nc.dram_tensor("maskm", [NK, NQ], bf16, kind="ExternalInput").ap()
    kones_d = nc.dram_tensor("kones", [NK, 1], bf16, kind="ExternalInput").ap()
    id_d = nc.dram_tensor("ident", [P, P], bf16, kind="ExternalInput").ap()
    out_d = nc.dram_tensor("out", [NQ, HQ], f32, kind="ExternalOutput").ap()
    rsd = nc.dram_tensor("rsd", [P, NQ], bf16)

    with tile.TileContext(nc) as tc, ExitStack() as ctx:
        const = ctx.enter_context(tc.tile_pool(name="const", bufs=1))
        persist = ctx.enter_context(tc.tile_pool(name="persist", bufs=1))
        xpool = ctx.enter_context(tc.tile_pool(name="xpool", bufs=2))
        wstream = ctx.enter_context(tc.tile_pool(name="wstream", bufs=2))
        work = ctx.enter_context(tc.tile_pool(name="work", bufs=2))
        ropetab = ctx.enter_context(tc.tile_pool(name="ropetab", bufs=1))
        pmpool = ctx.enter_context(tc.tile_pool(name="pmpool", bufs=10))
        rbpool = ctx.enter_context(tc.tile_pool(name="rbpool", bufs=4))
        pp = ctx.enter_context(tc.tile_pool(name="pp", bufs=2, space="PSUM"))
        psc = ctx.enter_context(tc.tile_pool(name="psc", bufs=3, space="PSUM"))
        pv = ctx.enter_context(tc.tile_pool(name="pv", bufs=3, space="PSUM"))

        # ---- initial loads: x/w chunks first so the PE starts ASAP ----
        xk_sb = xpool.tile([P, DT, NK], bf16, name="xsb")
        xk_r = xk_d.rearrange("(t p) q -> p t q", p=P)
        wkc = wstream.tile([P, DT, 512], bf16, name="wchunk")
        wk_r = wk_d.rearrange("(t p) o -> p t o", p=P)
        for a, b2 in [(0, 1), (1, 2), (2, 4), (4, 8), (8, 16)]:
            nc.sync.dma_start(xk_sb[:, a:b2, :], xk_r[:, a:b2, :])
            nc.sync.dma_start(wkc[:, a:b2, :], wk_r[:, a:b2, :])

        ident = const.tile([P, P], bf16, name="ident")
        nc.sync.dma_start(ident[:], id_d)
        ones64 = const.tile([1, HD], bf16, name="ones64")
        nc.vector.memset(ones64[:], 1.0)

        Qt = persist.tile([P, HQT, NQ], bf16, name="Qt")
        KtRz = persist.tile([P, KVH, NK], bf16, name="KtRz")
        Vaug = persist.tile([P, NKT, KVH, P], bf16, name="Vaug")
        attnT = persist.tile([P, HQT, NQ], bf16, name="attnT")
        mask_sb = persist.tile([P, NKT, NQ], bf16, name="mask_sb")

        nc.vector.memset(KtRz[64:128, 0:4, :], 0.0)
        nc.vector.memset(KtRz[0:64, 4:8, :], 0.0)
        nc.vector.memset(Vaug[:, :, :, HD:P], 0.0)
        kones_sb = const.tile([P, NKT], bf16, name="kones_sb")
        nc.sync.dma_start(kones_sb[:], kones_d.rearrange("(t p) o -> p (t o)", p=P))
        for kt in range(NKT):
            for g in range(KVH):
                nc.vector.tensor_copy(Vaug[:, kt, g, HD:HD + 1],
                                      kones_sb[:, kt:kt + 1])

        cosk = ropetab.tile([P, NKT, 512], bf16, name="cos")
        sink = ropetab.tile([P, NKT, 512], bf16, name="sin")
        nc.sync.dma_start(cosk[:], cosk_d.rearrange("(t p) c -> p t c", p=P))
        nc.sync.dma_start(sink[:], sink_d.rearrange("(t p) c -> p t c", p=P))
        nc.sync.dma_start(mask_sb[:], mask_d.rearrange("(t p) q -> p t q", p=P))

        def rope_block(ps, cos_t, sin_t, ti):
            nat = work.tile([P, 512], f32, name="nat")
            nc.vector.tensor_copy(nat[:], ps[:])
            ro = work.tile([P, 512], f32, name="ro")
            nc.gpsimd.tensor_mul(ro[:, 0::2], nat[:, 1::2], sin_t[:, ti, 0::2])
            nc.gpsimd.tensor_mul(ro[:, 1::2], nat[:, 0::2], sin_t[:, ti, 1::2])
            tmp = work.tile([P, 512], f32, name="tmp")
            nc.vector.tensor_mul(tmp[:], nat[:], cos_t[:, ti, :])
            rot = work.tile([P, 512], bf16, name="rot")
            nc.vector.tensor_add(rot[:], ro[:], tmp[:])
            return rot

        # ---- K projection + rope + transpose (zero-padded halves) ----
        # rope+transpose evictions run one tile behind the projection
        # matmuls so the PE never stalls on the rope chain
        def k_evict(ps, kt):
            rot = rope_block(ps, cosk, sink, kt)
            ks = slice(kt * P, (kt + 1) * P)
            for b in range(4):
                pst = psc.tile([P, P], bf16, name="psS")
                nc.tensor.transpose(pst[:], rot[:, b * P:(b + 1) * P], ident[:])
                half = (2 * b) // 4
                lo = half * 64
                nc.scalar.copy(KtRz[lo:lo + 64, 2 * b, ks], pst[0:64, :])
                nc.scalar.copy(KtRz[lo:lo + 64, 2 * b + 1, ks], pst[64:128, :])

        prevk = None
        for kt in range(NKT):
            ps = pp.tile([P, 512], f32, name="pj")
            for dt in range(DT):
                nc.tensor.matmul(ps[:], xk_sb[:, dt, kt * P:(kt + 1) * P],
                                 wkc[:, dt, :], start=(dt == 0),
                                 stop=(dt == DT - 1))
            if prevk is not None:
                k_evict(*prevk)
            prevk = (ps, kt)
        k_evict(*prevk)

        # ---- V projection -> Vaug ----
        wvc = wstream.tile([P, DT, 512], bf16, name="wchunk")
        nc.sync.dma_start(wvc[:], wv_d.rearrange("(t p) o -> p t o", p=P))
        for kt in range(NKT):
            ps = pp.tile([P, 512], f32, name="pj")
            for dt in range(DT):
                nc.tensor.matmul(ps[:], xk_sb[:, dt, kt * P:(kt + 1) * P],
                                 wvc[:, dt, :], start=(dt == 0),
                                 stop=(dt == DT - 1))
            nc.vector.tensor_copy(Vaug[:, kt, :, 0:HD],
                                  ps[:].rearrange("p (g d) -> p g d", g=KVH))

        # ---- Q projection + rope + transpose (head-permuted wq) ----
        cosq = ropetab.tile([P, NQT, 512], bf16, name="cos")
        sinq = ropetab.tile([P, NQT, 512], bf16, name="sin")
        nc.sync.dma_start(cosq[:], cosq_d.rearrange("(t p) c -> p t c", p=P))
        nc.sync.dma_start(sinq[:], sinq_d.rearrange("(t p) c -> p t c", p=P))
        xq_sb = xpool.tile([P, DT, NQ], bf16, name="xsb")
        nc.sync.dma_start(xq_sb[:], xq_d.rearrange("(t p) q -> p t q", p=P))
        def q_evict(ps, hc, qt):
            rot = rope_block(ps, cosq, sinq, qt)
            for b in range(4):
                pst = psc.tile([P, P], bf16, name="psS")
                nc.tensor.transpose(pst[:], rot[:, b * P:(b + 1) * P], ident[:])
                dst = Qt[:, hc * 4 + b, qt * P:(qt + 1) * P]
                if b % 2 == 0:
                    nc.scalar.copy(dst, pst[:])
                else:
                    nc.vector.tensor_copy(dst, pst[:])

        prevq = None
        for hc in range(HC):
            wqc = wstream.tile([P, DT, 512], bf16, name="wchunk")
            nc.sync.dma_start(
                wqc[:],
                wq_d[:, hc * 512:(hc + 1) * 512].rearrange("(t p) o -> p t o", p=P))
            for qt in range(NQT):
                ps = pp.tile([P, 512], f32, name="pj")
                for dt in range(DT):
                    nc.tensor.matmul(ps[:], xq_sb[:, dt, qt * P:(qt + 1) * P],
                                     wqc[:, dt, :], start=(dt == 0),
                                     stop=(dt == DT - 1))
                if prevq is not None:
                    q_evict(*prevq)
                prevq = (ps, hc, qt)
        q_evict(*prevq)

        # ---- attention per tile t = heads (t, 16+t) ----
        rs_all = persist.tile([P, NQ], f32, name="rs_all")
        rs_rcp = persist.tile([P, NQ], bf16, name="rs_rcp")

        def norm_pass(trange, rows):
            with nc.allow_low_precision(reason="softmax denominator in bf16"):
                nc.vector.reciprocal(rs_rcp[rows], rs_all[rows])
            # bounce the reciprocal rows through DRAM: a DRAM-source DMA may
            # use a stride-0 partition dim, giving a free 64-way broadcast
            nc.sync.dma_start(rsd.ap()[rows, :], rs_rcp[rows])
            for t2 in trange:
                for par in range(2):
                    h_lo = par * 64
                    r = (t2 // 4) * 32 + (t2 % 4) * 2 + par
                    rb = rbpool.tile([P, NQ], bf16, name="rb")
                    nc.sync.dma_start(
                        rb[h_lo:h_lo + 64, :],
                        rsd.ap()[r:r + 1, :].partition_broadcast(64).squeeze(1))
                    sl = attnT[h_lo:h_lo + 64, t2, :]
                    nc.vector.tensor_mul(sl, sl, rb[h_lo:h_lo + 64, :])

        stash = []
        for t in range(HQT):
            groups = (t // 4, 4 + t // 4)
            for (qc, qcw) in qchunks:
                live = [kt for kt in range(NKT)
                        if kt * P <= qc + qcw - 1 + offs_max]
                psO = [pv.tile([P, 512], f32, name="pvo")[:, :qcw]
                       for _ in range(2)]
                pms = {}

                def qk_exp_mask(kt, par):
                    lo = max(0, kt * P - qc - offs_max)
                    g = groups[par]
                    psS = psc.tile([P, 512], f32, name="psS")[:, :qcw]
                    nc.tensor.matmul(
                        psS[:, lo:], KtRz[:, g, kt * P:(kt + 1) * P],
                        Qt[:, t, qc + lo:qc + qcw], start=True, stop=True)
                    if causal:
                        pm = pmpool.tile([P, 512], bf16, name="pm")[:, :qcw]
                        nc.scalar.activation(pm[:, lo:], psS[:, lo:], EXPF,
                                             bias=0.0, scale=0.125)
                        d0 = kt * P - qc
                        dlo, dhi = max(lo, d0), min(qcw, d0 + P)
                        if dlo < dhi:
                            nc.vector.tensor_mul(
                                pm[:, dlo:dhi], pm[:, dlo:dhi],
                                mask_sb[:, kt, qc + dlo:qc + dhi])
                    else:
                        pexp = pmpool.tile([P, 512], bf16, name="pexp")[:, :qcw]
                        nc.scalar.activation(pexp[:, lo:], psS[:, lo:], EXPF,
                                             bias=0.0, scale=0.125)
                        pm = pmpool.tile([P, 512], bf16, name="pm")[:, :qcw]
                        nc.vector.tensor_mul(pm[:, lo:], pexp[:, lo:],
                                             mask_sb[:, kt, qc + lo:qc + qcw])
                    return pm, lo

                def pv_mm(idx):
                    kt = live[idx]
                    for par in range(2):
                        pm, lo = pms[(idx, par)]
                        nc.tensor.matmul(
                            psO[par][:, lo:], Vaug[:, kt, groups[par], :],
                            pm[:, lo:], start=(idx == 0),
                            stop=(idx == len(live) - 1), skip_group_check=True)

                for idx, kt in enumerate(live):
                    for par in range(2):
                        pms[(idx, par)] = qk_exp_mask(kt, par)
                    if idx > 0:
                        pv_mm(idx - 1)
                        del pms[(idx - 1, 0)], pms[(idx - 1, 1)]
                pv_mm(len(live) - 1)

                for par in range(2):
                    h_lo = par * 64
                    dst = attnT[h_lo:h_lo + 64, t, qc:qc + qcw]
                    nc.vector.tensor_copy(dst, psO[par][0:64, :])
                    rsum0 = work.tile([1, 512], f32, name="rsum0")[:, :qcw]
                    nc.vector.tensor_copy(rsum0, psO[par][64:65, :])
                    r = (t // 4) * 32 + (t % 4) * 2 + par
                    nc.sync.dma_start(rs_all[r:r + 1, qc:qc + qcw], rsum0)
            if t % 4 == 3:
                qi = t // 4
                norm_pass(range(qi * 4, qi * 4 + 4), slice(qi * 32, qi * 32 + 8))
            if t == 11:
                # head tiles 0-11 are final: start the first two output-
                # projection groups (j=0..11) inside the attention window
                woc0 = wstream.tile([P, DT, 512], bf16, name="wchunk")
                nc.sync.dma_start(
                    woc0[:], wo_d[:, 0:512].rearrange("(t p) o -> p t o", p=P))
                for qt0 in range(2):
                    ps0 = pp.tile([P, 512], f32, name="pj")
                    for j in range(12):
                        nc.tensor.matmul(ps0[:],
                                         attnT[:, j, qt0 * P:(qt0 + 1) * P],
                                         woc0[:, j, :], start=(j == 0),
                                         stop=False)
                    stash.append((ps0, woc0, 0, qt0))

        # ---- output projection (wo rows head-permuted) ----
        # the first two groups run contraction steps j=0..11 up front
        # (those head tiles are normalized early) so the PE has work while
        # the final quarter's normalization chain completes
        def wo_finish(ps, woc, dc, qt, jlo):
            for j in range(jlo, HQT):
                nc.tensor.matmul(ps[:], attnT[:, j, qt * P:(qt + 1) * P],
                                 woc[:, j, :], start=(j == 0),
                                 stop=(j == HQT - 1))
            osb = work.tile([P, 512], f32, name="osb")
            nc.vector.tensor_copy(osb[:], ps[:])
            nc.sync.dma_start(
                out_d[qt * P:(qt + 1) * P, dc * 512:(dc + 1) * 512], osb[:])

        for args in stash:
            wo_finish(*args, 12)
        for dc in range(4):
            if dc == 0:
                woc = stash[0][1]
            else:
                woc = wstream.tile([P, DT, 512], bf16, name="wchunk")
                nc.sync.dma_start(
                    woc[:], wo_d[:, dc * 512:(dc + 1) * 512]
                    .rearrange("(t p) o -> p t o", p=P))
            for qt in range(NQT):
                if dc == 0 and qt < 2:
                    continue
                ps = pp.tile([P, 512], f32, name="pj")
                wo_finish(ps, woc, dc, qt, 0)

    nc.finalize()
    _BUILD_CACHE[key] = nc
    return nc


# ---------------------------------------------------------------------------
# entry point
# ---------------------------------------------------------------------------

def kernel(x, freqs_cis, sequence_id, wq, wk, wv, wo):
    x = np.asarray(x, dtype=np.float32)
    freqs_cis = np.asarray(freqs_cis, dtype=np.float32)
    sequence_id = np.asarray(sequence_id)

    jobs = _plan_jobs(sequence_id)
    NQ = _round_up(max(max(j[2] for j in jobs), 1), P)
    NK = _round_up(max(max(j[1] + j[2] - j[3] for j in jobs), 1), P)
    offs_max = max(j[1] - j[3] for j in jobs)

    def single_doc(j):
        b, qs, ql, ks = j
        if ql == 0:
            return True
        seg = np.asarray(sequence_id[b])[ks:qs + ql]
        return bool((seg == seg[0]).all())

    causal = offs_max == 0 and all(single_doc(j) for j in jobs)

    cos_tab = freqs_cis[:, :, 0].astype(np.float32)
    sin_tab = freqs_cis[:, :, 1].astype(np.float32)
    wq_t = _permute_wq(np.ascontiguousarray(np.asarray(wq, np.float32).T)).astype(bf)
    wk_t = np.ascontiguousarray(np.asarray(wk, np.float32).T).astype(bf)
    wv_t = np.ascontiguousarray(np.asarray(wv, np.float32).T).astype(bf)
    wo_t = _permute_wo(np.ascontiguousarray(np.asarray(wo, np.float32).T)).astype(bf)
    id16 = np.eye(P, dtype=bf)

    in_maps = []
    for job in jobs:
        p = _core_inputs(job, NQ, NK, x, sequence_id, cos_tab, sin_tab)
        p.update({"wq_t": wq_t, "wk_t": wk_t, "wv_t": wv_t, "wo_t": wo_t,
                  "ident": id16})
        in_maps.append(p)

    nc = _build(NQ, NK, offs_max, causal)
    res = run_bass_kernel_spmd(nc, in_maps, core_ids=list(range(N_CORES)))

    full = np.zeros((BS, S, HQ), dtype=np.float32)
    for job, r in zip(jobs, res.results):
        b, qs, ql, ks = job
        if ql > 0:
            full[b, qs:qs + ql] = r["out"][:ql]
    return full



# revision 24
# speedup vs baseline: 1.0614x; 1.0614x over previous
"""Document-causal GQA attention on 8 TRN2 NeuronCores.

Strategy: the packed-document mask makes attention block-diagonal over
(batch, document) segments, so each of the 8 cores gets one segment's
queries together with its KV window — no cross-core communication.

v4 device kernel: all projections emit outputs directly in the layout
the next stage consumes ([head-dim, token] for Q/K via weight-stationary
matmuls, [token, v-dim] for V), eliminating every PE transpose.  RoPE is
applied in [dim, token] layout with a pair-swap permutation matmul
(T1 = Pswap @ T0) plus two elementwise muls and an add.  The Q
projection runs first so its ~60us of PE work hides all remaining input
DMA.  Attention batches the exp over both heads of a tile (one ACT call
per KV tile), applies the causal mask with gpsimd affine_select, and
keeps raw PV outputs + denominators; normalization happens in two
batched reciprocal passes with a DRAM-bounce partition broadcast (two
DMAs + one multiply per four head tiles).  The output projection is
weight-stationary too, producing out^T = [d_model, token] which the
host transposes back.  Moving operands are split into even halves so
LDWEIGHTS always hides under the previous matmul's stream.
"""
import numpy as np
import ml_dtypes

from contextlib import ExitStack

import concourse.bass as bass
import concourse.tile as tile
from concourse import bacc, mybir
from concourse.bass_utils import run_bass_kernel_spmd

BS, S, D, H, KVH, HD = 2, 2048, 2048, 32, 8, 64
N_REP = H // KVH
HQ = H * HD
HKV = KVH * HD
P = 128
N_CORES = 8
DT = D // P
HQT = HQ // P

f32 = mybir.dt.float32
bf16 = mybir.dt.bfloat16
EXPF = mybir.ActivationFunctionType.Exp
bf = ml_dtypes.bfloat16

# column order so that 128-col block t of wq holds heads (t, 16+t)
HEAD_ORDER = [i // 2 if i % 2 == 0 else 16 + i // 2 for i in range(32)]
# column order so that 128-col block j of wk holds kv groups (j, 4+j)
K_ORDER = [0, 4, 1, 5, 2, 6, 3, 7]


# ---------------------------------------------------------------------------
# host-side planning
# ---------------------------------------------------------------------------

def _round_up(x, m):
    return ((x + m - 1) // m) * m


def _even_chunks(n):
    if n <= 512:
        return [(0, n)]
    h = _round_up(n // 2, 32)
    return [(0, h), (h, n - h)]


def _plan_jobs(sequence_id):
    jobs = []
    for b in range(BS):
        sid = np.asarray(sequence_id[b])
        starts = [0] + list(np.where(np.diff(sid) != 0)[0] + 1) + [len(sid)]
        for i in range(len(starts) - 1):
            jobs.append([b, int(starts[i]), int(starts[i + 1] - starts[i]),
                         int(starts[i])])
    while len(jobs) > N_CORES:
        best, bi = None, -1
        for i in range(len(jobs) - 1):
            a, c = jobs[i], jobs[i + 1]
            if a[0] == c[0] and a[1] + a[2] == c[1]:
                cost = (c[1] + c[2]) - min(a[3], c[3])
                if best is None or cost < best:
                    best, bi = cost, i
        a, c = jobs[bi], jobs[bi + 1]
        jobs[bi] = [a[0], a[1], a[2] + c[2], min(a[3], c[3])]
        del jobs[bi + 1]
    while len(jobs) < N_CORES:
        i = max(range(len(jobs)), key=lambda j: jobs[j][2])
        b, qs, ql, ks = jobs[i]
        if ql < 2:
            jobs.append([b, qs, 0, qs])
            continue
        h = ql // 2
        jobs[i] = [b, qs, h, ks]
        jobs.insert(i + 1, [b, qs + h, ql - h, ks])
    return jobs


def _permute_wq(wq_t):
    return np.ascontiguousarray(
        wq_t.reshape(D, 32, 64)[:, HEAD_ORDER, :].reshape(D, HQ))


def _permute_wk(wk_t):
    return np.ascontiguousarray(
        wk_t.reshape(D, KVH, HD)[:, K_ORDER, :].reshape(D, HKV))


def _permute_wo(wo_t):
    return np.ascontiguousarray(
        wo_t.reshape(32, 64, D)[HEAD_ORDER].reshape(HQ, D))


def _pswap():
    m = np.zeros((P, P), dtype=np.float32)
    for i in range(P):
        m[i ^ 1, i] = 1.0
    return m.astype(bf)


def _rope_tables(cos_tab, sin_tab, start, ln, n):
    # [128, n] tables in [dim, token] layout: row r covers pair (r % 64)//2
    # of either head in the 2-head 128-row tile; sin rows carry the rope
    # signs (-s on even rows, +s on odd rows).  Padded tokens get identity.
    pair = (np.arange(P) % HD) // 2
    sign = np.where(np.arange(P) % 2 == 0, -1.0, 1.0).astype(np.float32)
    cosT = np.ones((P, n), dtype=np.float32)
    sinT = np.zeros((P, n), dtype=np.float32)
    c = cos_tab[start:start + ln]  # [ln, 32]
    s = sin_tab[start:start + ln]
    cosT[:, :ln] = c[:, pair].T
    sinT[:, :ln] = s[:, pair].T * sign[:, None]
    return cosT.astype(bf), sinT.astype(bf)


def _core_inputs(job, NQ, NK, x, sequence_id, cos_tab, sin_tab, causal):
    b, qs, ql, ks = job
    kl = qs + ql - ks
    NKT = (NK + P - 1) // P

    xq_t = np.zeros((D, NQ), dtype=np.float32)
    xq_t[:, :ql] = x[b, qs:qs + ql].T
    xk_t = np.zeros((D, NK), dtype=np.float32)
    xk_t[:, :kl] = x[b, ks:ks + kl].T

    cos_q, sin_q = _rope_tables(cos_tab, sin_tab, qs, ql, NQ)
    cos_k, sin_k = _rope_tables(cos_tab, sin_tab, ks, kl, NK)

    sid = np.asarray(sequence_id[b])
    sid_q = np.full(NQ, -2, dtype=np.int64)
    sid_q[:ql] = sid[qs:qs + ql]
    sid_k = np.full(NK, -1, dtype=np.int64)
    sid_k[:kl] = sid[ks:ks + kl]
    gq = qs + np.arange(NQ)
    gk = ks + np.arange(NK)
    mfull = ((sid_k[:, None] == sid_q[None, :]) &
             (gk[:, None] <= gq[None, :])).astype(np.float32)
    if causal:
        # only diagonal 128x128 blocks are ever masked on-device
        mask = np.zeros((P, NKT, P), dtype=np.float32)
        for kt in range(NKT):
            r0 = kt * P
            rs_ = min(P, NK - r0)
            cs_ = min(P, NQ - r0)
            mask[:rs_, kt, :cs_] = mfull[r0:r0 + rs_, r0:r0 + cs_]
    else:
        # padded query columns attend to key 0 so denominators stay finite
        mfull[0, ql:] = 1.0
        mask = np.ascontiguousarray(
            mfull.reshape(NKT, P, NQ).transpose(1, 0, 2))
    kones = np.zeros((NKT * P, 1), dtype=np.float32)
    kones[:kl] = 1.0

    return {
        "xq_t": xq_t.astype(bf), "xk_t": xk_t.astype(bf),
        "cos_q": cos_q, "sin_q": sin_q, "cos_k": cos_k, "sin_k": sin_k,
        "maskm": mask.astype(bf), "kones": kones.astype(bf),
    }


def _common_inputs(wq, wk, wv, wo):
    wq_t = _permute_wq(np.ascontiguousarray(np.asarray(wq, np.float32).T))
    wk_t = _permute_wk(np.ascontiguousarray(np.asarray(wk, np.float32).T))
    wv_t = np.ascontiguousarray(np.asarray(wv, np.float32).T)
    wo_t = _permute_wo(np.ascontiguousarray(np.asarray(wo, np.float32).T))
    return {"wq_t": wq_t.astype(bf), "wk_t": wk_t.astype(bf),
            "wv_t": wv_t.astype(bf), "wo_t": wo_t.astype(bf),
            "pswap": _pswap()}


# ---------------------------------------------------------------------------
# device graph
# ---------------------------------------------------------------------------

_BUILD_CACHE = {}


def _build(NQ, NK, offs_max, causal):
    key = (NQ, NK, offs_max, causal)
    if key in _BUILD_CACHE:
        return _BUILD_CACHE[key]
    NKT = (NK + P - 1) // P
    # attention query chunks: a 512-wide psum chunk plus a small batched
    # tail; when the tail would overflow one psum bank, use an even split
    if NQ <= 512:
        qchunks = [(0, NQ)]
    elif NQ - 512 <= 64:
        qchunks = [(0, 512), (512, NQ - 512)]
    else:
        qchunks = _even_chunks(NQ)
    # projection / output moving chunks: even halves keep every matmul's
    # stream long enough to hide the next LDWEIGHTS
    pq = _even_chunks(NQ)
    pk = _even_chunks(NK)
    kw = [min(P, NK - kt * P) for kt in range(NKT)]  # kv-tile widths

    nc = bacc.Bacc("TRN2", target_bir_lowering=False, debug=False,
                   num_devices=N_CORES)

    xq_d = nc.dram_tensor("xq_t", [D, NQ], bf16, kind="ExternalInput").ap()
    xk_d = nc.dram_tensor("xk_t", [D, NK], bf16, kind="ExternalInput").ap()
    wq_d = nc.dram_tensor("wq_t", [D, HQ], bf16, kind="ExternalInput").ap()
    wk_d = nc.dram_tensor("wk_t", [D, HKV], bf16, kind="ExternalInput").ap()
    wv_d = nc.dram_tensor("wv_t", [D, HKV], bf16, kind="ExternalInput").ap()
    wo_d = nc.dram_tensor("wo_t", [HQ, D], bf16, kind="ExternalInput").ap()
    cosq_d = nc.dram_tensor("cos_q", [P, NQ], bf16, kind="ExternalInput").ap()
    sinq_d = nc.dram_tensor("sin_q", [P, NQ], bf16, kind="ExternalInput").ap()
    cosk_d = nc.dram_tensor("cos_k", [P, NK], bf16, kind="ExternalInput").ap()
    sink_d = nc.dram_tensor("sin_k", [P, NK], bf16, kind="ExternalInput").ap()
    if causal:
        mask_d = nc.dram_tensor("maskm", [P, NKT, P], bf16,
                                kind="ExternalInput").ap()
    else:
        mask_d = nc.dram_tensor("maskm", [P, NKT, NQ], bf16,
                                kind="ExternalInput").ap()
    kones_d = nc.dram_tensor("kones", [NKT * P, 1], bf16,
                             kind="ExternalInput").ap()
    psw_d = nc.dram_tensor("pswap", [P, P], bf16, kind="ExternalInput").ap()
    out_d = nc.dram_tensor("out", [D, NQ], f32, kind="ExternalOutput").ap()
    rsd = nc.dram_tensor("rsd", [32, NQ], bf16)

    with tile.TileContext(nc) as tc, ExitStack() as ctx:
        const = ctx.enter_context(tc.tile_pool(name="const", bufs=1))
        persist = ctx.enter_context(tc.tile_pool(name="persist", bufs=1))
        xpool = ctx.enter_context(tc.tile_pool(name="xpool", bufs=1))
        wstream = ctx.enter_context(tc.tile_pool(name="wstream", bufs=2))
        work = ctx.enter_context(tc.tile_pool(name="work", bufs=6))
        opool = ctx.enter_context(tc.tile_pool(name="opool", bufs=2))
        ropetab = ctx.enter_context(tc.tile_pool(name="ropetab", bufs=1))
        pmpool = ctx.enter_context(tc.tile_pool(name="pmpool", bufs=4))
        pm2pool = ctx.enter_context(tc.tile_pool(name="pm2pool", bufs=2))
        rbpool = ctx.enter_context(tc.tile_pool(name="rbpool", bufs=4))

        # ---- initial DMAs: Q-side first (the Q projection starts within
        # ~2us and its PE work then hides every remaining input DMA) ----
        xq_sb = xpool.tile([P, DT, NQ], bf16, name="xqsb")
        xq_r = xq_d.rearrange("(t p) q -> p t q", p=P)
        pswap = const.tile([P, P], bf16, name="pswap")
        cosq = ropetab.tile([P, NQ], bf16, name="cosq")
        sinq = ropetab.tile([P, NQ], bf16, name="sinq")
        wqcs = [wstream.tile([P, DT, P], bf16, name="wqc") for _ in range(2)]
        nc.sync.dma_start(xq_sb[:, 0:1, :], xq_r[:, 0:1, :])
        nc.sync.dma_start(
            wqcs[0][:], wq_d[:, 0:P].rearrange("(t p) o -> p t o", p=P))
        for a, b2 in [(1, 2), (2, 4), (4, 8), (8, 16)]:
            nc.sync.dma_start(xq_sb[:, a:b2, :], xq_r[:, a:b2, :])
        nc.sync.dma_start(pswap[:], psw_d)
        nc.sync.dma_start(cosq[:], cosq_d)
        nc.sync.dma_start(sinq[:], sinq_d)
        nc.sync.dma_start(
            wqcs[1][:], wq_d[:, P:2 * P].rearrange("(t p) o -> p t o", p=P))

        # K/V-side inputs stream in underneath the Q projection
        xk_sb = xpool.tile([P, DT, NK], bf16, name="xksb")
        xk_r = xk_d.rearrange("(t p) k -> p t k", p=P)
        wv_sb = xpool.tile([P, DT, HKV], bf16, name="wvsb")
        wv_r = wv_d.rearrange("(t p) o -> p t o", p=P)
        for a, b2 in [(0, 4), (4, 8), (8, 16)]:
            nc.sync.dma_start(xk_sb[:, a:b2, :], xk_r[:, a:b2, :])
            nc.sync.dma_start(wv_sb[:, a:b2, :], wv_r[:, a:b2, :])
        wk_sb = xpool.tile([P, DT, HKV], bf16, name="wksb")
        nc.sync.dma_start(wk_sb[:], wk_d.rearrange("(t p) o -> p t o", p=P))
        cosk = ropetab.tile([P, NK], bf16, name="cosk")
        sink = ropetab.tile([P, NK], bf16, name="sink")
        nc.sync.dma_start(cosk[:], cosk_d)
        nc.sync.dma_start(sink[:], sink_d)
        kones_sb = const.tile([P, NKT], bf16, name="kones_sb")
        nc.sync.dma_start(kones_sb[:],
                          kones_d.rearrange("(t p) o -> p (t o)", p=P))
        if causal:
            mask_sb = None  # diagonal masking runs via gpsimd affine_select
        else:
            mask_sb = persist.tile([P, NKT, NQ], bf16, name="mask_sb")
            nc.sync.dma_start(mask_sb[:], mask_d)

        Qt = persist.tile([P, HQT, NQ], bf16, name="Qt")
        KtRz = persist.tile([P, KVH, NK], bf16, name="KtRz")
        Vaug = persist.tile([P, NKT, KVH, P], bf16, name="Vaug")
        attnT = persist.tile([P, HQT, NQ], bf16, name="attnT")
        rs_all = persist.tile([32, NQ], f32, name="rs_all")
        rs_rcp = persist.tile([32, NQ], bf16, name="rs_rcp")

        # rows 24:32 are read by the first reciprocal pass before they are
        # written; keep them finite
        nc.vector.memset(rs_all[:], 1.0)
        nc.vector.memset(KtRz[64:128, 0:4, :], 0.0)
        nc.vector.memset(KtRz[0:64, 4:8, :], 0.0)
        nc.vector.memset(Vaug[:, :, :, HD:P], 0.0)
        for kt in range(NKT):
            for g in range(KVH):
                nc.vector.tensor_copy(Vaug[:, kt, g, HD:HD + 1],
                                      kones_sb[:, kt:kt + 1])

        # ---- rope in [dim, token] layout: rot = T0*cos + (Pswap@T0)*sin ----
        def rope_chunks(ps_list, chunks, cosT, sinT, dests):
            for (c0, cwd), psc in zip(chunks, ps_list):
                t0sb = work.tile([P, 512], bf16, name="t0sb")[:, :cwd]
                nc.scalar.copy(t0sb, psc[:])
                t1 = t1pool.tile([P, 512], f32, name="t1")[:, :cwd]
                nc.tensor.matmul(t1, pswap[:], t0sb, start=True, stop=True)
                tmp = work.tile([P, 512], bf16, name="tmp")[:, :cwd]
                nc.vector.tensor_mul(tmp, t0sb, cosT[:, c0:c0 + cwd])
                cross = work.tile([P, 512], bf16, name="cross")[:, :cwd]
                nc.vector.tensor_mul(cross, t1, sinT[:, c0:c0 + cwd])
                for (rlo, rhi, dst) in dests:
                    nc.vector.tensor_add(dst(c0, cwd), tmp[rlo:rhi],
                                         cross[rlo:rhi])

        # ================= Q + V + K projections =================
        # psum budget (8 banks): pj 3 + t1 3 + pv 2
        with tc.tile_pool(name="ppA", bufs=3, space="PSUM") as ppA, \
             tc.tile_pool(name="t1pool_", bufs=3, space="PSUM") as t1pool:

            # ---- Q projection (weight stationary -> [dim, token]) ----
            for t in range(HQT):
                if t < 2:
                    wqc = wqcs[t]
                else:
                    wqc = wstream.tile([P, DT, P], bf16, name="wqc")
                    nc.sync.dma_start(
                        wqc[:],
                        wq_d[:, t * P:(t + 1) * P]
                        .rearrange("(t p) o -> p t o", p=P))
                pss = []
                for (c0, cwd) in pq:
                    ps = ppA.tile([P, 512], f32, name="pj")[:, :cwd]
                    for dt in range(DT):
                        nc.tensor.matmul(ps, wqc[:, dt, :],
                                         xq_sb[:, dt, c0:c0 + cwd],
                                         start=(dt == 0), stop=(dt == DT - 1))
                    pss.append(ps)
                rope_chunks(
                    pss, pq, cosq, sinq,
                    [(0, 128, lambda c0, cwd, t=t:
                      Qt[:, t, c0:c0 + cwd])])

            # ---- V projection (x-tile stationary) ----
            for kt in range(NKT):
                ps = t1pool.tile([P, 512], f32, name="pv",
                                 bufs=2)[:kw[kt], :]
                for dt in range(DT):
                    nc.tensor.matmul(ps, xk_sb[:, dt, kt * P:kt * P + kw[kt]],
                                     wv_sb[:, dt, :], start=(dt == 0),
                                     stop=(dt == DT - 1))
                nc.scalar.copy(
                    Vaug[:kw[kt], kt, :, 0:HD],
                    ps.rearrange("p (g d) -> p g d", g=KVH))

            # ---- K projection (weight stationary -> [dim, token]) ----
            for j in range(4):
                pss = []
                for (c0, cwd) in pk:
                    ps = ppA.tile([P, 512], f32, name="pj")[:, :cwd]
                    for dt in range(DT):
                        nc.tensor.matmul(ps, wk_sb[:, dt, j * P:(j + 1) * P],
                                         xk_sb[:, dt, c0:c0 + cwd],
                                         start=(dt == 0), stop=(dt == DT - 1))
                    pss.append(ps)
                rope_chunks(
                    pss, pk, cosk, sink,
                    [(0, 64, lambda c0, cwd, j=j:
                      KtRz[0:64, j, c0:c0 + cwd]),
                     (64, 128, lambda c0, cwd, j=j:
                      KtRz[64:128, 4 + j, c0:c0 + cwd])])

        # ================= attention =================
        def norm_pass(trange, rows):
            # reciprocal over all 32 rows (engine partition bases must be
            # 32-aligned); only `rows` of the result are fresh/forwarded
            with nc.allow_low_precision(reason="softmax denominator in bf16"):
                nc.vector.reciprocal(rs_rcp[0:32], rs_all[0:32])
            # bounce reciprocals through DRAM: a DRAM-source DMA may use a
            # stride-0 partition dim, giving a free 64-way broadcast.  Two
            # DMAs + one mul cover four head tiles at a time.
            nc.sync.dma_start(rsd.ap()[rows, :], rs_rcp[rows])
            ts = list(trange)
            for i in range(0, len(ts), 4):
                grp = ts[i:i + 4]
                t0, ng = grp[0], len(grp)
                rb = rbpool.tile([P, 4, NQ], bf16, name="rb")[:, :ng, :]
                nc.sync.dma_start(
                    rb[0:64],
                    rsd.ap()[2 * t0:2 * t0 + 2 * ng:2, :]
                    .partition_broadcast(64))
                nc.sync.dma_start(
                    rb[64:128],
                    rsd.ap()[2 * t0 + 1:2 * t0 + 2 * ng:2, :]
                    .partition_broadcast(64))
                sl = attnT[:, t0:t0 + ng, :]
                nc.vector.tensor_mul(sl, sl, rb)

        with tc.tile_pool(name="psS", bufs=2, space="PSUM") as psSp, \
             tc.tile_pool(name="psS2", bufs=1, space="PSUM") as psS2p, \
             tc.tile_pool(name="psO", bufs=1, space="PSUM") as psOp:

            fill0 = nc.gpsimd.to_reg(0.0)

            def diag_mask(pm_sl, dw, base):
                nc.gpsimd.affine_select(
                    pm_sl, pm_sl, pattern=[[0, 2], [1, dw]],
                    compare_op=mybir.AluOpType.is_ge, fill=fill0,
                    base=base, channel_multiplier=-1)

            def evict(t, qc, qcw, psO_par, dns):
                for par in range(2):
                    h_lo = par * 64
                    nc.vector.tensor_copy(
                        attnT[h_lo:h_lo + 64, t, qc:qc + qcw],
                        psO_par(par)[0:64, :])
                    nc.vector.tensor_copy(dns[par][:, qc:qc + qcw],
                                          psO_par(par)[64:65, :])

            def attn_chunk_main(t, qc, qcw, groups, dns):
                live = [kt for kt in range(NKT)
                        if kt * P <= qc + qcw - 1 + offs_max]
                psO = psOp.tile([P, 2, 512], f32, name="psO")[:, :, :qcw]
                pms = []
                for idx, kt in enumerate(live):
                    lo = max(0, kt * P - qc - offs_max)
                    psS = psSp.tile([P, 2, 512], f32, name="psS")[:, :, :qcw]
                    for par in range(2):
                        nc.tensor.matmul(
                            psS[:kw[kt], par, lo:],
                            KtRz[:, groups[par], kt * P:kt * P + kw[kt]],
                            Qt[:, t, qc + lo:qc + qcw],
                            start=True, stop=True)
                    pm = pmpool.tile([P, 2, 512], bf16, name="pm")[:, :, :qcw]
                    nc.scalar.activation(pm[:kw[kt], :, lo:],
                                         psS[:kw[kt], :, lo:], EXPF,
                                         bias=0.0, scale=0.125)
                    d0 = kt * P - qc
                    dlo, dhi = max(lo, d0), min(qcw, d0 + P)
                    if causal:
                        if dlo < dhi:
                            diag_mask(pm[:kw[kt], :, dlo:dhi], dhi - dlo,
                                      qc + dlo - kt * P)
                    else:
                        for par in range(2):
                            nc.vector.tensor_mul(
                                pm[:kw[kt], par, lo:], pm[:kw[kt], par, lo:],
                                mask_sb[:kw[kt], kt, qc + lo:qc + qcw])
                    pms.append((kt, lo, pm))
                    # PV one tile behind so exp/mask overlap the next QK
                    if idx > 0:
                        kp, lp, pmp = pms[idx - 1]
                        for par in range(2):
                            nc.tensor.matmul(
                                psO[:, par, lp:], Vaug[:kw[kp], kp,
                                                       groups[par], :],
                                pmp[:kw[kp], par, lp:], start=(idx == 1),
                                stop=False, skip_group_check=True)
                kp, lp, pmp = pms[-1]
                for par in range(2):
                    nc.tensor.matmul(
                        psO[:, par, lp:],
                        Vaug[:kw[kp], kp, groups[par], :],
                        pmp[:kw[kp], par, lp:], start=(len(live) == 1),
                        stop=True, skip_group_check=True)
                evict(t, qc, qcw, lambda par: psO[:, par, :], dns)

            def attn_chunk_tail(t, qc, qcw, groups, dns):
                # small ragged tail: batch all KV tiles' scores into one
                # 1-bank psum tile, slots laid out along the free dim only
                live = [kt for kt in range(NKT)
                        if kt * P <= qc + qcw - 1 + offs_max]
                L = len(live)
                nfull = len([kt for kt in live if kw[kt] == P])
                psS2 = psS2p.tile([P, 2, L * qcw], f32, name="psS2")
                psO2 = psOp.tile([P, 2, 512], f32, name="psO")[:, :, :qcw]
                for si, kt in enumerate(live):
                    for par in range(2):
                        nc.tensor.matmul(
                            psS2[:kw[kt], par, si * qcw:(si + 1) * qcw],
                            KtRz[:, groups[par], kt * P:kt * P + kw[kt]],
                            Qt[:, t, qc:qc + qcw], start=True, stop=True)
                pm2 = pm2pool.tile([P, 2, L * qcw], bf16, name="pm2")
                if nfull:
                    nc.scalar.activation(pm2[:, :, :nfull * qcw],
                                         psS2[:, :, :nfull * qcw], EXPF,
                                         bias=0.0, scale=0.125)
                for si in range(nfull, L):
                    kt = live[si]
                    nc.scalar.activation(
                        pm2[:kw[kt], :, si * qcw:(si + 1) * qcw],
                        psS2[:kw[kt], :, si * qcw:(si + 1) * qcw], EXPF,
                        bias=0.0, scale=0.125)
                for si, kt in enumerate(live):
                    d0, d1 = kt * P - qc, kt * P + P - qc
                    dlo, dhi = max(0, d0), min(qcw, d1)
                    if causal:
                        if dlo < dhi:
                            diag_mask(
                                pm2[:kw[kt], :, si * qcw + dlo:si * qcw + dhi],
                                dhi - dlo, qc + dlo - kt * P)
                    else:
                        for par in range(2):
                            nc.vector.tensor_mul(
                                pm2[:kw[kt], par, si * qcw:(si + 1) * qcw],
                                pm2[:kw[kt], par, si * qcw:(si + 1) * qcw],
                                mask_sb[:kw[kt], kt, qc:qc + qcw])
                for si, kt in enumerate(live):
                    for par in range(2):
                        nc.tensor.matmul(
                            psO2[:, par, :],
                            Vaug[:kw[kt], kt, groups[par], :],
                            pm2[:kw[kt], par, si * qcw:(si + 1) * qcw],
                            start=(si == 0), stop=(si == L - 1),
                            skip_group_check=True)
                evict(t, qc, qcw, lambda par: psO2[:, par, :], dns)

            for t in range(HQT):
                groups = (t // 4, 4 + t // 4)
                dns = [work.tile([1, NQ], f32, name="dn") for _ in range(2)]
                for (qc, qcw) in qchunks:
                    if qcw > 64:
                        attn_chunk_main(t, qc, qcw, groups, dns)
                    else:
                        attn_chunk_tail(t, qc, qcw, groups, dns)
                # engine APs need 32-aligned partition bases; route each
                # denominator row to its rs_all slot through one DMA
                for par in range(2):
                    nc.sync.dma_start(
                        rs_all[2 * t + par:2 * t + par + 1, :], dns[par][:])
                if t == 11:
                    norm_pass(range(0, 12), slice(0, 24))

            # prefetch the first two output-projection weight tiles so the
            # PE can start Wo the moment normalization completes
            wocs = [wstream.tile([P, HQT, P], bf16, name="woc")
                    for _ in range(2)]
            for dc in range(2):
                nc.sync.dma_start(
                    wocs[dc][:],
                    wo_d[:, dc * P:(dc + 1) * P]
                    .rearrange("(j p) c -> p j c", p=P))
            norm_pass(range(12, 16), slice(24, 32))

        # ========== output projection (weight stationary) ==========
        with tc.tile_pool(name="psW", bufs=4, space="PSUM") as psWp:
            for dc in range(DT):
                if dc < 2:
                    woc = wocs[dc]
                else:
                    woc = wstream.tile([P, HQT, P], bf16, name="woc")
                    nc.sync.dma_start(
                        woc[:],
                        wo_d[:, dc * P:(dc + 1) * P]
                        .rearrange("(j p) c -> p j c", p=P))
                osb = opool.tile([P, NQ], f32, name="osb")
                for (c0, cwd) in pq:
                    ps = psWp.tile([P, 512], f32, name="pw")[:, :cwd]
                    for j in range(HQT):
                        nc.tensor.matmul(ps, woc[:, j, :],
                                         attnT[:, j, c0:c0 + cwd],
                                         start=(j == 0), stop=(j == HQT - 1))
                    nc.vector.tensor_copy(osb[:, c0:c0 + cwd], ps)
                nc.sync.dma_start(out_d[dc * P:(dc + 1) * P, :], osb[:])

    nc.finalize()
    _BUILD_CACHE[key] = nc
    return nc


# ---------------------------------------------------------------------------
# entry point
# ---------------------------------------------------------------------------

def kernel(x, freqs_cis, sequence_id, wq, wk, wv, wo):
    x = np.asarray(x, dtype=np.float32)
    freqs_cis = np.asarray(freqs_cis, dtype=np.float32)
    sequence_id = np.asarray(sequence_id)

    jobs = _plan_jobs(sequence_id)
    NQ = _round_up(max(max(j[2] for j in jobs), 32), 32)
    NK = _round_up(max(max(j[1] + j[2] - j[3] for j in jobs), 32), 32)
    offs_max = max(j[1] - j[3] for j in jobs)

    def single_doc(j):
        b, qs, ql, ks = j
        if ql == 0:
            return True
        seg = np.asarray(sequence_id[b])[ks:qs + ql]
        return bool((seg == seg[0]).all())

    causal = offs_max == 0 and all(single_doc(j) for j in jobs)

    cos_tab = freqs_cis[:, :, 0].astype(np.float32)
    sin_tab = freqs_cis[:, :, 1].astype(np.float32)
    common = _common_inputs(wq, wk, wv, wo)

    in_maps = []
    for job in jobs:
        p = _core_inputs(job, NQ, NK, x, sequence_id, cos_tab, sin_tab,
                         causal)
        p.update(common)
        in_maps.append(p)

    nc = _build(NQ, NK, offs_max, causal)
    res = run_bass_kernel_spmd(nc, in_maps, core_ids=list(range(N_CORES)))

    full = np.zeros((BS, S, HQ), dtype=np.float32)
    for job, r in zip(jobs, res.results):
        b, qs, ql, ks = job
        if ql > 0:
            full[b, qs:qs + ql] = r["out"][:, :ql].T
    return full


# revision 32
# speedup vs baseline: 1.1868x; 1.1181x over previous
"""Document-causal GQA attention on 8 TRN2 NeuronCores.

Strategy: the packed-document mask makes attention block-diagonal over
(batch, document) segments, so each of the 8 cores gets one segment's
queries together with its KV window — no cross-core communication.

v4 device kernel: all projections emit outputs directly in the layout
the next stage consumes ([head-dim, token] for Q/K via weight-stationary
matmuls, [token, v-dim] for V), eliminating every PE transpose.  RoPE is
applied in [dim, token] layout with a pair-swap permutation matmul
(T1 = Pswap @ T0) plus two elementwise muls and an add.  The Q
projection runs first so its ~60us of PE work hides all remaining input
DMA.  Attention batches the exp over both heads of a tile (one ACT call
per KV tile), applies the causal mask with gpsimd affine_select, and
keeps raw PV outputs + denominators; normalization happens in two
batched reciprocal passes with a DRAM-bounce partition broadcast (two
DMAs + one multiply per four head tiles).  The output projection is
weight-stationary too, producing out^T = [d_model, token] which the
host transposes back.  Moving operands are split into even halves so
LDWEIGHTS always hides under the previous matmul's stream.
"""
import numpy as np
import ml_dtypes

from contextlib import ExitStack

import concourse.bass as bass
import concourse.tile as tile
from concourse import bacc, mybir
from concourse.bass_utils import run_bass_kernel_spmd

BS, S, D, H, KVH, HD = 2, 2048, 2048, 32, 8, 64
N_REP = H // KVH
HQ = H * HD
HKV = KVH * HD
P = 128
N_CORES = 8
DT = D // P
HQT = HQ // P

f32 = mybir.dt.float32
bf16 = mybir.dt.bfloat16
EXPF = mybir.ActivationFunctionType.Exp
bf = ml_dtypes.bfloat16

# column order so that 128-col block t of wq holds heads (t, 16+t)
HEAD_ORDER = [i // 2 if i % 2 == 0 else 16 + i // 2 for i in range(32)]
# column order so that 128-col block j of wk holds kv groups (j, 4+j)
K_ORDER = [0, 4, 1, 5, 2, 6, 3, 7]


# ---------------------------------------------------------------------------
# host-side planning
# ---------------------------------------------------------------------------

def _round_up(x, m):
    return ((x + m - 1) // m) * m


def _even_chunks(n):
    if n <= 512:
        return [(0, n)]
    h = _round_up(n // 2, 32)
    return [(0, h), (h, n - h)]


def _plan_jobs(sequence_id):
    jobs = []
    for b in range(BS):
        sid = np.asarray(sequence_id[b])
        starts = [0] + list(np.where(np.diff(sid) != 0)[0] + 1) + [len(sid)]
        for i in range(len(starts) - 1):
            jobs.append([b, int(starts[i]), int(starts[i + 1] - starts[i]),
                         int(starts[i])])
    while len(jobs) > N_CORES:
        best, bi = None, -1
        for i in range(len(jobs) - 1):
            a, c = jobs[i], jobs[i + 1]
            if a[0] == c[0] and a[1] + a[2] == c[1]:
                cost = (c[1] + c[2]) - min(a[3], c[3])
                if best is None or cost < best:
                    best, bi = cost, i
        a, c = jobs[bi], jobs[bi + 1]
        jobs[bi] = [a[0], a[1], a[2] + c[2], min(a[3], c[3])]
        del jobs[bi + 1]
    while len(jobs) < N_CORES:
        i = max(range(len(jobs)), key=lambda j: jobs[j][2])
        b, qs, ql, ks = jobs[i]
        if ql < 2:
            jobs.append([b, qs, 0, qs])
            continue
        h = ql // 2
        jobs[i] = [b, qs, h, ks]
        jobs.insert(i + 1, [b, qs + h, ql - h, ks])
    return jobs


def _permute_wq(wq_t):
    return np.ascontiguousarray(
        wq_t.reshape(D, 32, 64)[:, HEAD_ORDER, :].reshape(D, HQ))


def _permute_wk(wk_t):
    return np.ascontiguousarray(
        wk_t.reshape(D, KVH, HD)[:, K_ORDER, :].reshape(D, HKV))


def _permute_wo(wo_t):
    return np.ascontiguousarray(
        wo_t.reshape(32, 64, D)[HEAD_ORDER].reshape(HQ, D))


def _pswap():
    m = np.zeros((P, P), dtype=np.float32)
    for i in range(P):
        m[i ^ 1, i] = 1.0
    return m.astype(bf)


def _rope_tables(cos_tab, sin_tab, start, ln, n):
    # [128, n] tables in [dim, token] layout: row r covers pair (r % 64)//2
    # of either head in the 2-head 128-row tile; sin rows carry the rope
    # signs (-s on even rows, +s on odd rows).  Padded tokens get identity.
    pair = (np.arange(P) % HD) // 2
    sign = np.where(np.arange(P) % 2 == 0, -1.0, 1.0).astype(np.float32)
    cosT = np.ones((P, n), dtype=np.float32)
    sinT = np.zeros((P, n), dtype=np.float32)
    c = cos_tab[start:start + ln]  # [ln, 32]
    s = sin_tab[start:start + ln]
    cosT[:, :ln] = c[:, pair].T
    sinT[:, :ln] = s[:, pair].T * sign[:, None]
    return cosT.astype(bf), sinT.astype(bf)


def _core_inputs(job, NQ, NK, x, sequence_id, cos_tab, sin_tab, causal):
    b, qs, ql, ks = job
    kl = qs + ql - ks
    NKT = (NK + P - 1) // P

    xq_t = np.zeros((D, NQ), dtype=np.float32)
    xq_t[:, :ql] = x[b, qs:qs + ql].T
    xk_t = np.zeros((D, NK), dtype=np.float32)
    xk_t[:, :kl] = x[b, ks:ks + kl].T

    cos_q, sin_q = _rope_tables(cos_tab, sin_tab, qs, ql, NQ)
    cos_k, sin_k = _rope_tables(cos_tab, sin_tab, ks, kl, NK)

    sid = np.asarray(sequence_id[b])
    sid_q = np.full(NQ, -2, dtype=np.int64)
    sid_q[:ql] = sid[qs:qs + ql]
    sid_k = np.full(NK, -1, dtype=np.int64)
    sid_k[:kl] = sid[ks:ks + kl]
    gq = qs + np.arange(NQ)
    gk = ks + np.arange(NK)
    mfull = ((sid_k[:, None] == sid_q[None, :]) &
             (gk[:, None] <= gq[None, :])).astype(np.float32)
    if causal:
        # only diagonal 128x128 blocks are ever masked on-device
        mask = np.zeros((P, NKT, P), dtype=np.float32)
        for kt in range(NKT):
            r0 = kt * P
            rs_ = min(P, NK - r0)
            cs_ = min(P, NQ - r0)
            mask[:rs_, kt, :cs_] = mfull[r0:r0 + rs_, r0:r0 + cs_]
    else:
        # padded query columns attend to key 0 so denominators stay finite
        mfull[0, ql:] = 1.0
        mask = np.ascontiguousarray(
            mfull.reshape(NKT, P, NQ).transpose(1, 0, 2))
    kones = np.zeros((NKT * P, 1), dtype=np.float32)
    kones[:kl] = 1.0

    return {
        "xq_t": xq_t.astype(bf), "xk_t": xk_t.astype(bf),
        "cos_q": cos_q, "sin_q": sin_q, "cos_k": cos_k, "sin_k": sin_k,
        "maskm": mask.astype(bf), "kones": kones.astype(bf),
    }


def _common_inputs(wq, wk, wv, wo):
    wq_t = _permute_wq(np.ascontiguousarray(np.asarray(wq, np.float32).T))
    wk_t = _permute_wk(np.ascontiguousarray(np.asarray(wk, np.float32).T))
    wv_t = np.ascontiguousarray(np.asarray(wv, np.float32).T)
    wo_t = _permute_wo(np.ascontiguousarray(np.asarray(wo, np.float32).T))
    return {"wq_t": wq_t.astype(bf), "wk_t": wk_t.astype(bf),
            "wv_t": wv_t.astype(bf), "wo_t": wo_t.astype(bf),
            "pswap": _pswap()}


# ---------------------------------------------------------------------------
# device graph
# ---------------------------------------------------------------------------

_BUILD_CACHE = {}


def _build(NQ, NK, offs_max, causal):
    key = (NQ, NK, offs_max, causal)
    if key in _BUILD_CACHE:
        return _BUILD_CACHE[key]
    NKT = (NK + P - 1) // P
    # attention query chunks: a 512-wide psum chunk plus a small batched
    # tail; when the tail would overflow one psum bank, use an even split
    if NQ <= 512:
        qchunks = [(0, NQ)]
    elif NQ - 512 <= 64:
        qchunks = [(0, 512), (512, NQ - 512)]
    else:
        qchunks = _even_chunks(NQ)
    # projection / output moving chunks: even halves keep every matmul's
    # stream long enough to hide the next LDWEIGHTS
    pq = _even_chunks(NQ)
    pk = _even_chunks(NK)
    kw = [min(P, NK - kt * P) for kt in range(NKT)]  # kv-tile widths

    nc = bacc.Bacc("TRN2", target_bir_lowering=False, debug=False,
                   num_devices=N_CORES)

    xq_d = nc.dram_tensor("xq_t", [D, NQ], bf16, kind="ExternalInput").ap()
    xk_d = nc.dram_tensor("xk_t", [D, NK], bf16, kind="ExternalInput").ap()
    wq_d = nc.dram_tensor("wq_t", [D, HQ], bf16, kind="ExternalInput").ap()
    wk_d = nc.dram_tensor("wk_t", [D, HKV], bf16, kind="ExternalInput").ap()
    wv_d = nc.dram_tensor("wv_t", [D, HKV], bf16, kind="ExternalInput").ap()
    wo_d = nc.dram_tensor("wo_t", [HQ, D], bf16, kind="ExternalInput").ap()
    cosq_d = nc.dram_tensor("cos_q", [P, NQ], bf16, kind="ExternalInput").ap()
    sinq_d = nc.dram_tensor("sin_q", [P, NQ], bf16, kind="ExternalInput").ap()
    cosk_d = nc.dram_tensor("cos_k", [P, NK], bf16, kind="ExternalInput").ap()
    sink_d = nc.dram_tensor("sin_k", [P, NK], bf16, kind="ExternalInput").ap()
    if causal:
        mask_d = nc.dram_tensor("maskm", [P, NKT, P], bf16,
                                kind="ExternalInput").ap()
    else:
        mask_d = nc.dram_tensor("maskm", [P, NKT, NQ], bf16,
                                kind="ExternalInput").ap()
    kones_d = nc.dram_tensor("kones", [NKT * P, 1], bf16,
                             kind="ExternalInput").ap()
    psw_d = nc.dram_tensor("pswap", [P, P], bf16, kind="ExternalInput").ap()
    out_d = nc.dram_tensor("out", [D, NQ], f32, kind="ExternalOutput").ap()
    rsd = nc.dram_tensor("rsd", [32, NQ], bf16)

    with tile.TileContext(nc) as tc, ExitStack() as ctx:
        const = ctx.enter_context(tc.tile_pool(name="const", bufs=1))
        persist = ctx.enter_context(tc.tile_pool(name="persist", bufs=1))
        xpool = ctx.enter_context(tc.tile_pool(name="xpool", bufs=1))
        wstream = ctx.enter_context(tc.tile_pool(name="wstream", bufs=2))
        work = ctx.enter_context(tc.tile_pool(name="work", bufs=6))
        opool = ctx.enter_context(tc.tile_pool(name="opool", bufs=2))
        ropetab = ctx.enter_context(tc.tile_pool(name="ropetab", bufs=1))
        pmpool = ctx.enter_context(tc.tile_pool(name="pmpool", bufs=4))
        pm2pool = ctx.enter_context(tc.tile_pool(name="pm2pool", bufs=2))
        rbpool = ctx.enter_context(tc.tile_pool(name="rbpool", bufs=2))

        # ---- initial DMAs: Q-side first (the Q projection starts within
        # ~2us and its PE work then hides every remaining input DMA) ----
        xq_sb = xpool.tile([P, DT, NQ], bf16, name="xqsb")
        xq_r = xq_d.rearrange("(t p) q -> p t q", p=P)
        pswap = const.tile([P, P], bf16, name="pswap")
        cosq = ropetab.tile([P, NQ], bf16, name="cosq")
        sinq = ropetab.tile([P, NQ], bf16, name="sinq")
        wqcs = [wstream.tile([P, DT, P], bf16, name="wqc") for _ in range(2)]
        nc.sync.dma_start(xq_sb[:, 0:1, :], xq_r[:, 0:1, :])
        nc.sync.dma_start(
            wqcs[0][:], wq_d[:, 0:P].rearrange("(t p) o -> p t o", p=P))
        for a, b2 in [(1, 2), (2, 4), (4, 8), (8, 16)]:
            nc.sync.dma_start(xq_sb[:, a:b2, :], xq_r[:, a:b2, :])
        nc.sync.dma_start(pswap[:], psw_d)
        nc.sync.dma_start(cosq[:], cosq_d)
        nc.sync.dma_start(sinq[:], sinq_d)
        nc.sync.dma_start(
            wqcs[1][:], wq_d[:, P:2 * P].rearrange("(t p) o -> p t o", p=P))

        # K/V-side inputs stream in underneath the Q projection
        xk_sb = xpool.tile([P, DT, NK], bf16, name="xksb")
        xk_r = xk_d.rearrange("(t p) k -> p t k", p=P)
        wv_sb = xpool.tile([P, DT, HKV], bf16, name="wvsb")
        wv_r = wv_d.rearrange("(t p) o -> p t o", p=P)
        for a, b2 in [(0, 4), (4, 8), (8, 16)]:
            nc.sync.dma_start(xk_sb[:, a:b2, :], xk_r[:, a:b2, :])
            nc.sync.dma_start(wv_sb[:, a:b2, :], wv_r[:, a:b2, :])
        wk_sb = xpool.tile([P, DT, HKV], bf16, name="wksb")
        nc.sync.dma_start(wk_sb[:], wk_d.rearrange("(t p) o -> p t o", p=P))
        cosk = ropetab.tile([P, NK], bf16, name="cosk")
        sink = ropetab.tile([P, NK], bf16, name="sink")
        nc.sync.dma_start(cosk[:], cosk_d)
        nc.sync.dma_start(sink[:], sink_d)
        kones_sb = const.tile([P, NKT], bf16, name="kones_sb")
        nc.sync.dma_start(kones_sb[:],
                          kones_d.rearrange("(t p) o -> p (t o)", p=P))
        if causal:
            mask_sb = None  # diagonal masking runs via gpsimd affine_select
        else:
            mask_sb = persist.tile([P, NKT, NQ], bf16, name="mask_sb")
            nc.sync.dma_start(mask_sb[:], mask_d)

        Qt = persist.tile([P, HQT, NQ], bf16, name="Qt")
        KtRz = persist.tile([P, KVH, NK], bf16, name="KtRz")
        Vaug = persist.tile([P, NKT, KVH, P], bf16, name="Vaug")
        attnT = persist.tile([P, HQT, NQ], bf16, name="attnT")
        rs_all = persist.tile([32, NQ], f32, name="rs_all")
        rs_rcp = persist.tile([32, NQ], bf16, name="rs_rcp")

        # rows 24:32 are read by the first reciprocal pass before they are
        # written; keep them finite
        nc.vector.memset(rs_all[:], 1.0)
        nc.vector.memset(KtRz[64:128, 0:4, :], 0.0)
        nc.vector.memset(KtRz[0:64, 4:8, :], 0.0)
        nc.vector.memset(Vaug[:, :, :, HD:P], 0.0)
        for kt in range(NKT):
            for g in range(KVH):
                nc.vector.tensor_copy(Vaug[:, kt, g, HD:HD + 1],
                                      kones_sb[:, kt:kt + 1])

        # ---- rope in [dim, token] layout: rot = T0*cos + (Pswap@T0)*sin ----
        def rope_chunks(ps_list, chunks, cosT, sinT, dests):
            for (c0, cwd), psc in zip(chunks, ps_list):
                t0sb = work.tile([P, 512], bf16, name="t0sb")[:, :cwd]
                nc.scalar.copy(t0sb, psc[:])
                t1 = t1pool.tile([P, 512], f32, name="t1")[:, :cwd]
                nc.tensor.matmul(t1, pswap[:], t0sb, start=True, stop=True)
                tmp = work.tile([P, 512], bf16, name="tmp")[:, :cwd]
                nc.vector.tensor_mul(tmp, t0sb, cosT[:, c0:c0 + cwd])
                cross = work.tile([P, 512], bf16, name="cross")[:, :cwd]
                nc.vector.tensor_mul(cross, t1, sinT[:, c0:c0 + cwd])
                for (rlo, rhi, dst) in dests:
                    nc.vector.tensor_add(dst(c0, cwd), tmp[rlo:rhi],
                                         cross[rlo:rhi])

        # ================= Q + V + K projections =================
        # psum budget (8 banks): pj 3 + t1 3 + pv 2
        with tc.tile_pool(name="ppA", bufs=3, space="PSUM") as ppA, \
             tc.tile_pool(name="t1pool_", bufs=3, space="PSUM") as t1pool:

            # ---- Q projection (weight stationary -> [dim, token]) ----
            for t in range(HQT):
                if t < 2:
                    wqc = wqcs[t]
                else:
                    wqc = wstream.tile([P, DT, P], bf16, name="wqc")
                    nc.sync.dma_start(
                        wqc[:],
                        wq_d[:, t * P:(t + 1) * P]
                        .rearrange("(t p) o -> p t o", p=P))
                pss = []
                for (c0, cwd) in pq:
                    ps = ppA.tile([P, 512], f32, name="pj")[:, :cwd]
                    for dt in range(DT):
                        nc.tensor.matmul(ps, wqc[:, dt, :],
                                         xq_sb[:, dt, c0:c0 + cwd],
                                         start=(dt == 0), stop=(dt == DT - 1))
                    pss.append(ps)
                rope_chunks(
                    pss, pq, cosq, sinq,
                    [(0, 128, lambda c0, cwd, t=t:
                      Qt[:, t, c0:c0 + cwd])])

            # ---- V projection (x-tile stationary) ----
            for kt in range(NKT):
                ps = t1pool.tile([P, 512], f32, name="pv",
                                 bufs=2)[:kw[kt], :]
                for dt in range(DT):
                    nc.tensor.matmul(ps, xk_sb[:, dt, kt * P:kt * P + kw[kt]],
                                     wv_sb[:, dt, :], start=(dt == 0),
                                     stop=(dt == DT - 1))
                nc.scalar.copy(
                    Vaug[:kw[kt], kt, :, 0:HD],
                    ps.rearrange("p (g d) -> p g d", g=KVH))

            # ---- K projection (weight stationary -> [dim, token]) ----
            for j in range(4):
                pss = []
                for (c0, cwd) in pk:
                    ps = ppA.tile([P, 512], f32, name="pj")[:, :cwd]
                    for dt in range(DT):
                        nc.tensor.matmul(ps, wk_sb[:, dt, j * P:(j + 1) * P],
                                         xk_sb[:, dt, c0:c0 + cwd],
                                         start=(dt == 0), stop=(dt == DT - 1))
                    pss.append(ps)
                rope_chunks(
                    pss, pk, cosk, sink,
                    [(0, 64, lambda c0, cwd, j=j:
                      KtRz[0:64, j, c0:c0 + cwd]),
                     (64, 128, lambda c0, cwd, j=j:
                      KtRz[64:128, 4 + j, c0:c0 + cwd])])

        # ================= attention =================
        def norm_pass(trange, rows):
            # reciprocal over all 32 rows (engine partition bases must be
            # 32-aligned); only `rows` of the result are fresh/forwarded
            with nc.allow_low_precision(reason="softmax denominator in bf16"):
                nc.vector.reciprocal(rs_rcp[0:32], rs_all[0:32])
            # bounce reciprocals through DRAM: a DRAM-source DMA may use a
            # stride-0 partition dim, giving a free 64-way broadcast.  Two
            # DMAs + one mul cover four head tiles at a time.
            nc.sync.dma_start(rsd.ap()[rows, :], rs_rcp[rows])
            ts = list(trange)
            for i in range(0, len(ts), 4):
                grp = ts[i:i + 4]
                t0, ng = grp[0], len(grp)
                rb = rbpool.tile([P, 4, NQ], bf16, name="rb")[:, :ng, :]
                nc.sync.dma_start(
                    rb[0:64],
                    rsd.ap()[2 * t0:2 * t0 + 2 * ng:2, :]
                    .partition_broadcast(64))
                nc.sync.dma_start(
                    rb[64:128],
                    rsd.ap()[2 * t0 + 1:2 * t0 + 2 * ng:2, :]
                    .partition_broadcast(64))
                sl = attnT[:, t0:t0 + ng, :]
                nc.vector.tensor_mul(sl, sl, rb)

        # psum: psS 4 banks + psO 2 + psS2 1 + psO2 1 = 8
        with tc.tile_pool(name="psS", bufs=2, space="PSUM") as psSp, \
             tc.tile_pool(name="psS2", bufs=1, space="PSUM") as psS2p, \
             tc.tile_pool(name="psO", bufs=1, space="PSUM") as psOp, \
             tc.tile_pool(name="psO2", bufs=1, space="PSUM") as psO2p:

            fill0 = nc.gpsimd.to_reg(0.0)

            def diag_mask(pm_sl, dw, base):
                nc.gpsimd.affine_select(
                    pm_sl, pm_sl, pattern=[[0, 2], [1, dw]],
                    compare_op=mybir.AluOpType.is_ge, fill=fill0,
                    base=base, channel_multiplier=-1)

            def evict(t, qc, qcw, psO_par, dns):
                for par in range(2):
                    h_lo = par * 64
                    nc.vector.tensor_copy(
                        attnT[h_lo:h_lo + 64, t, qc:qc + qcw],
                        psO_par(par)[0:64, :])
                    nc.vector.tensor_copy(dns[par][:, qc:qc + qcw],
                                          psO_par(par)[64:65, :])

            def attn_chunk_main(t, qc, qcw, groups, dns):
                live = [kt for kt in range(NKT)
                        if kt * P <= qc + qcw - 1 + offs_max]
                psO = psOp.tile([P, 2, 512], f32, name="psO")[:, :, :qcw]
                pms = []
                for idx, kt in enumerate(live):
                    lo = max(0, kt * P - qc - offs_max)
                    psS = psSp.tile([P, 2, 512], f32, name="psS")[:, :, :qcw]
                    for par in range(2):
                        nc.tensor.matmul(
                            psS[:kw[kt], par, lo:],
                            KtRz[:, groups[par], kt * P:kt * P + kw[kt]],
                            Qt[:, t, qc + lo:qc + qcw],
                            start=True, stop=True)
                    pm = pmpool.tile([P, 2, 512], bf16, name="pm")[:, :, :qcw]
                    nc.scalar.activation(pm[:kw[kt], :, lo:],
                                         psS[:kw[kt], :, lo:], EXPF,
                                         bias=0.0, scale=0.125)
                    d0 = kt * P - qc
                    dlo, dhi = max(lo, d0), min(qcw, d0 + P)
                    if causal:
                        if dlo < dhi:
                            diag_mask(pm[:kw[kt], :, dlo:dhi], dhi - dlo,
                                      qc + dlo - kt * P)
                    else:
                        for par in range(2):
                            nc.vector.tensor_mul(
                                pm[:kw[kt], par, lo:], pm[:kw[kt], par, lo:],
                                mask_sb[:kw[kt], kt, qc + lo:qc + qcw])
                    pms.append((kt, lo, pm))
                    # PV one tile behind so exp/mask overlap the next QK
                    if idx > 0:
                        kp, lp, pmp = pms[idx - 1]
                        for par in range(2):
                            nc.tensor.matmul(
                                psO[:, par, lp:], Vaug[:kw[kp], kp,
                                                       groups[par], :],
                                pmp[:kw[kp], par, lp:], start=(idx == 1),
                                stop=False, skip_group_check=True)
                kp, lp, pmp = pms[-1]
                for par in range(2):
                    nc.tensor.matmul(
                        psO[:, par, lp:],
                        Vaug[:kw[kp], kp, groups[par], :],
                        pmp[:kw[kp], par, lp:], start=(len(live) == 1),
                        stop=True, skip_group_check=True)
                evict(t, qc, qcw, lambda par: psO[:, par, :], dns)

            def attn_chunk_tail(t, qc, qcw, groups, dns):
                # small ragged tail: batch all KV tiles' scores into one
                # 1-bank psum tile, slots laid out along the free dim only
                live = [kt for kt in range(NKT)
                        if kt * P <= qc + qcw - 1 + offs_max]
                L = len(live)
                nfull = len([kt for kt in live if kw[kt] == P])
                psS2 = psS2p.tile([P, 2, L * qcw], f32, name="psS2")
                psO2 = psO2p.tile([P, 2, qcw], f32, name="psO2")
                for si, kt in enumerate(live):
                    for par in range(2):
                        nc.tensor.matmul(
                            psS2[:kw[kt], par, si * qcw:(si + 1) * qcw],
                            KtRz[:, groups[par], kt * P:kt * P + kw[kt]],
                            Qt[:, t, qc:qc + qcw], start=True, stop=True)
                pm2 = pm2pool.tile([P, 2, L * qcw], bf16, name="pm2")
                if nfull:
                    nc.scalar.activation(pm2[:, :, :nfull * qcw],
                                         psS2[:, :, :nfull * qcw], EXPF,
                                         bias=0.0, scale=0.125)
                for si in range(nfull, L):
                    kt = live[si]
                    nc.scalar.activation(
                        pm2[:kw[kt], :, si * qcw:(si + 1) * qcw],
                        psS2[:kw[kt], :, si * qcw:(si + 1) * qcw], EXPF,
                        bias=0.0, scale=0.125)
                for si, kt in enumerate(live):
                    d0, d1 = kt * P - qc, kt * P + P - qc
                    dlo, dhi = max(0, d0), min(qcw, d1)
                    if causal:
                        if dlo < dhi:
                            diag_mask(
                                pm2[:kw[kt], :, si * qcw + dlo:si * qcw + dhi],
                                dhi - dlo, qc + dlo - kt * P)
                    else:
                        for par in range(2):
                            nc.vector.tensor_mul(
                                pm2[:kw[kt], par, si * qcw:(si + 1) * qcw],
                                pm2[:kw[kt], par, si * qcw:(si + 1) * qcw],
                                mask_sb[:kw[kt], kt, qc:qc + qcw])
                # par-outer: the 1-bank psO2 can hold only one OPEN psum
                # accumulation group at a time — close par0's before par1's
                for par in range(2):
                    for si, kt in enumerate(live):
                        nc.tensor.matmul(
                            psO2[:, par, :],
                            Vaug[:kw[kt], kt, groups[par], :],
                            pm2[:kw[kt], par, si * qcw:(si + 1) * qcw],
                            start=(si == 0), stop=(si == L - 1),
                            skip_group_check=True)
                evict(t, qc, qcw, lambda par: psO2[:, par, :], dns)

            for t in range(HQT):
                groups = (t // 4, 4 + t // 4)
                dns = [work.tile([1, NQ], f32, name="dn", bufs=8)
                       for _ in range(2)]
                for (qc, qcw) in qchunks:
                    if qcw > 64:
                        attn_chunk_main(t, qc, qcw, groups, dns)
                    else:
                        attn_chunk_tail(t, qc, qcw, groups, dns)
                # engine APs need 32-aligned partition bases; route each
                # denominator row to its rs_all slot through one DMA
                for par in range(2):
                    nc.sync.dma_start(
                        rs_all[2 * t + par:2 * t + par + 1, :], dns[par][:])
                if t == 11:
                    norm_pass(range(0, 12), slice(0, 24))
                if t == 14:
                    # prefetch the first two output-projection weight tiles
                    # while the SP queue is quiet
                    wocs = [wstream.tile([P, HQT, P], bf16, name="woc")
                            for _ in range(2)]
                    for dc in range(2):
                        nc.sync.dma_start(
                            wocs[dc][:],
                            wo_d[:, dc * P:(dc + 1) * P]
                            .rearrange("(j p) c -> p j c", p=P))

        # ========== output projection (weight stationary) ==========
        with tc.tile_pool(name="psW", bufs=4, space="PSUM") as psWp:
            # accumulate the first 12 head tiles of dc=0,1 while the final
            # normalization chain (reciprocal/broadcast/mul) runs
            held = []
            for dc in range(2):
                pss = []
                for (c0, cwd) in pq:
                    ps = psWp.tile([P, 512], f32, name="pw")[:, :cwd]
                    for j in range(12):
                        nc.tensor.matmul(ps, wocs[dc][:, j, :],
                                         attnT[:, j, c0:c0 + cwd],
                                         start=(j == 0), stop=False)
                    pss.append(ps)
                held.append(pss)
            norm_pass(range(12, 16), slice(24, 32))
            for dc in range(DT):
                if dc < 2:
                    woc = wocs[dc]
                    pss = held[dc]
                    jlo = 12
                else:
                    woc = wstream.tile([P, HQT, P], bf16, name="woc")
                    nc.sync.dma_start(
                        woc[:],
                        wo_d[:, dc * P:(dc + 1) * P]
                        .rearrange("(j p) c -> p j c", p=P))
                    pss = [psWp.tile([P, 512], f32, name="pw")[:, :cwd]
                           for (c0, cwd) in pq]
                    jlo = 0
                osb = opool.tile([P, NQ], f32, name="osb")
                for (c0, cwd), ps in zip(pq, pss):
                    for j in range(jlo, HQT):
                        nc.tensor.matmul(ps, woc[:, j, :],
                                         attnT[:, j, c0:c0 + cwd],
                                         start=(j == 0), stop=(j == HQT - 1))
                    nc.vector.tensor_copy(osb[:, c0:c0 + cwd], ps)
                nc.sync.dma_start(out_d[dc * P:(dc + 1) * P, :], osb[:])

    nc.finalize()
    _BUILD_CACHE[key] = nc
    return nc


# ---------------------------------------------------------------------------
# entry point
# ---------------------------------------------------------------------------

def kernel(x, freqs_cis, sequence_id, wq, wk, wv, wo):
    x = np.asarray(x, dtype=np.float32)
    freqs_cis = np.asarray(freqs_cis, dtype=np.float32)
    sequence_id = np.asarray(sequence_id)

    jobs = _plan_jobs(sequence_id)
    NQ = _round_up(max(max(j[2] for j in jobs), 32), 32)
    NK = _round_up(max(max(j[1] + j[2] - j[3] for j in jobs), 32), 32)
    offs_max = max(j[1] - j[3] for j in jobs)

    def single_doc(j):
        b, qs, ql, ks = j
        if ql == 0:
            return True
        seg = np.asarray(sequence_id[b])[ks:qs + ql]
        return bool((seg == seg[0]).all())

    causal = offs_max == 0 and all(single_doc(j) for j in jobs)

    cos_tab = freqs_cis[:, :, 0].astype(np.float32)
    sin_tab = freqs_cis[:, :, 1].astype(np.float32)
    common = _common_inputs(wq, wk, wv, wo)

    in_maps = []
    for job in jobs:
        p = _core_inputs(job, NQ, NK, x, sequence_id, cos_tab, sin_tab,
                         causal)
        p.update(common)
        in_maps.append(p)

    nc = _build(NQ, NK, offs_max, causal)
    res = run_bass_kernel_spmd(nc, in_maps, core_ids=list(range(N_CORES)))

    full = np.zeros((BS, S, HQ), dtype=np.float32)
    for job, r in zip(jobs, res.results):
        b, qs, ql, ks = job
        if ql > 0:
            full[b, qs:qs + ql] = r["out"][:, :ql].T
    return full
